# revision 13
# baseline (speedup 1.0000x reference)
"""Trainium2 Bass kernel for nn_DeblendingTransformerBlock_18708877541727.

Sharding: head-parallel across 8 cores. Head i's attention output maps exactly
to output windows [128*i, 128*(i+1)) == contiguous spatial rows [32*i, 32*i+32)
for both batches, so each core owns head i end-to-end (qkv -> attention ->
proj -> LN1 -> MLP -> LN2) with zero cross-core communication.

Key algebraic folds (host-side, exact):
 - The v-projection and the output projection commute: per 64-channel d-group
   g (== output window g), proj_out = A @ (x_w @ (W_v_slice^T @ proj_w^T)),
   so the proj matmul weights fold into the qkv v-weights (vpW).
 - q scaling (SCALE) folds into the q-side qkv weights.
 - qkv/proj biases ride along as a ones-row (K=65) in the x^T operand.
 - norm1_b folds into the shipped shortcut.
"""

import numpy as np
import ml_dtypes
from contextlib import ExitStack

import concourse.bacc as bacc
import concourse.mybir as mybir
import concourse.tile as tile
from concourse.bass_utils import run_bass_kernel_spmd

F32 = mybir.dt.float32
BF16 = mybir.dt.bfloat16
FP16 = mybir.dt.float16
MMDT = FP16
MMNP = np.float16
AF = mybir.ActivationFunctionType
ALU = mybir.AluOpType

B = 2
C = 64
NH = 8
S = 64
NW = 1024
D = 8192          # per-head feature dim = NW*C/NH
HID = 256
EPS = 1e-5
SCALE = float((3 * (C // NH)) ** 0.5)
NG = 128          # 64-channel groups per head-tensor (q, k, or v)
NCORES = 8

_BUILD_CACHE = {}


def _rel_index():
    coords = np.stack(np.meshgrid(np.arange(8), np.arange(8), indexing='ij'))
    cf = coords.reshape(2, -1)
    rel = (cf[:, :, None] - cf[:, None, :]).transpose(1, 2, 0).copy()
    rel[..., 0] += 7
    rel[..., 1] += 7
    rel[..., 0] *= 15
    return rel.sum(-1)  # (64, 64)


def _bf(a):
    return np.ascontiguousarray(a.astype(MMNP))


def _f32(a):
    return np.ascontiguousarray(a.astype(np.float32))


def _build(flags):
    """Build the SPMD program. flags = (nobias, has_g1, has_g2, has_n2b, has_fc2b)."""
    nobias, has_g1, has_g2, has_n2b, has_fc2b = flags
    nc = bacc.Bacc(None)

    xTv_d = nc.dram_tensor("xTv", [65, 16384], MMDT, kind="ExternalInput")
    if nobias:
        xqk2_d = nc.dram_tensor("xqk2", [128, 2 * 16384], MMDT,
                                kind="ExternalInput")
        qw1_d = nc.dram_tensor("qw1", [64, NG * 64], MMDT, kind="ExternalInput")
        qw2_d = nc.dram_tensor("qw2", [128, NG * 64], MMDT, kind="ExternalInput")
        kw1_d = nc.dram_tensor("kw1", [64, NG * 64], MMDT, kind="ExternalInput")
        kw2_d = nc.dram_tensor("kw2", [128, NG * 64], MMDT, kind="ExternalInput")
    else:
        xqk_d = nc.dram_tensor("xqk", [65, 2 * 16384], MMDT,
                               kind="ExternalInput")
        qwT_d = nc.dram_tensor("qwT", [65, NG * 64], MMDT, kind="ExternalInput")
        kwT_d = nc.dram_tensor("kwT", [65, NG * 64], MMDT, kind="ExternalInput")
    vpW_d = nc.dram_tensor("vpW", [65, NG * 64], MMDT, kind="ExternalInput")
    id128_d = nc.dram_tensor("id128", [128, 128], MMDT, kind="ExternalInput")
    id64_d = nc.dram_tensor("id64", [64, 64], F32, kind="ExternalInput")
    battn_d = nc.dram_tensor("battn", [64, 64], F32, kind="ExternalInput")
    fc1wT_d = nc.dram_tensor("fc1wT", [64, 256], MMDT, kind="ExternalInput")
    fc1b_d = nc.dram_tensor("fc1b", [128, 2], F32, kind="ExternalInput")
    fc2wT_d = nc.dram_tensor("fc2wT", [128, 128], MMDT, kind="ExternalInput")
    sc_d = nc.dram_tensor("sc", [B, 8192, 64], F32, kind="ExternalInput")
    if has_g1:
        g1bc_d = nc.dram_tensor("g1bc", [128, 64], F32, kind="ExternalInput")
    if has_g2:
        g2bc_d = nc.dram_tensor("g2bc", [128, 64], F32, kind="ExternalInput")
    if has_n2b:
        n2bc_d = nc.dram_tensor("n2bc", [128, 64], F32, kind="ExternalInput")
    if has_fc2b:
        fc2bc_d = nc.dram_tensor("fc2bc", [128, 64], F32, kind="ExternalInput")
    out_d = nc.dram_tensor("out", [B, 8192, 64], F32, kind="ExternalOutput")

    # out/sc token row = 64*w + s = 128*q + p  (w = 2*q + p//64, s = p%64)
    sc_v = sc_d[:].rearrange("b (t q p) c -> b t p q c", t=8, q=8, p=128)
    out_v = out_d[:].rearrange("b (t q p) c -> b t p q c", t=8, q=8, p=128)

    with tile.TileContext(nc) as tc, ExitStack() as st:
        const = st.enter_context(tc.tile_pool(name="const", bufs=1))
        pers = st.enter_context(tc.tile_pool(name="pers", bufs=1))

        epsc = const.tile([128, 1], F32)
        nc.vector.memset(epsc[:], EPS)
        id128 = const.tile([128, 128], MMDT)
        id64 = const.tile([64, 64], F32)
        battn = const.tile([64, 64], F32)
        fc1w = const.tile([64, 256], MMDT)
        fc1b = const.tile([128, 2], F32)
        fc2w = const.tile([128, 128], MMDT)
        nc.sync.dma_start(id128[:], id128_d[:])
        nc.sync.dma_start(id64[:], id64_d[:])
        nc.sync.dma_start(battn[:], battn_d[:])
        nc.sync.dma_start(fc1w[:], fc1wT_d[:])
        nc.sync.dma_start(fc1b[:], fc1b_d[:])
        nc.sync.dma_start(fc2w[:], fc2wT_d[:])
        if has_g1:
            g1bc = const.tile([128, 64], F32)
            nc.sync.dma_start(g1bc[:], g1bc_d[:])
        if has_g2:
            g2bc = const.tile([128, 64], F32)
            nc.sync.dma_start(g2bc[:], g2bc_d[:])
        if has_n2b:
            n2bc = const.tile([128, 64], F32)
            nc.sync.dma_start(n2bc[:], n2bc_d[:])
        if has_fc2b:
            fc2bc = const.tile([128, 64], F32)
            nc.sync.dma_start(fc2bc[:], fc2bc_d[:])

        # persistent attention operands (both batches)
        vp = pers.tile([128, 8192], MMDT)   # rows 64b+t, cols 64w+oc
        AT = pers.tile([128, 64], MMDT)     # rows 64b+t, cols s

        # ---------------- QKV phase ----------------
        # v-phase: vp = per-group (x_w | ones) @ [Wv_slice^T @ proj_w^T ; b]
        with tc.tile_pool(name="xtv", bufs=1) as xtvp, \
             tc.tile_pool(name="vps", bufs=4, space="PSUM") as vpsp:
            xtv = xtvp.tile([65, 16384], MMDT)
            vw = xtvp.tile([65, NG * 64], MMDT)
            nc.sync.dma_start(xtv[:], xTv_d[:])
            nc.sync.dma_start(vw[:], vpW_d[:])
            for bank in range(16):
                ps = vpsp.tile([128, 512], F32, tag="ps")
                for gg in range(8):
                    g = 8 * bank + gg
                    for b in range(B):
                        nc.tensor.matmul(
                            ps[64 * b:64 * b + 64, 64 * gg:64 * gg + 64],
                            xtv[:, 128 * g + 64 * b:128 * g + 64 * b + 64],
                            vw[:, 64 * g:64 * g + 64],
                            start=True, stop=True)
                nc.vector.tensor_copy(vp[:, 512 * bank:512 * bank + 512], ps[:])

        # qk-phase: qT / kT (f32): out (64 d-rows, 128 = [b0 s | b1 s])
        with tc.tile_pool(name="qkbuf", bufs=1) as qkbuf:
            qT = qkbuf.tile([128, 8192], F32)
            kT = qkbuf.tile([128, 8192], F32)
            with tc.tile_pool(name="xtqk", bufs=1) as xtqkp, \
                 tc.tile_pool(name="qkps", bufs=4, space="PSUM") as qkpsp:
                if nobias:
                    xqk2 = xtqkp.tile([128, 2 * 16384], MMDT)
                    nc.sync.dma_start(xqk2[:], xqk2_d[:])
                else:
                    xqk = xtqkp.tile([65, 2 * 16384], MMDT)
                    nc.sync.dma_start(xqk[:], xqk_d[:])
                for ti, (dstT, eng) in enumerate(((qT, nc.scalar),
                                                  (kT, nc.vector))):
                    with tc.tile_pool(name=f"qkw{ti}", bufs=1) as qkwp:
                        if nobias:
                            w1 = qkwp.tile([64, NG * 64], MMDT, tag="w1")
                            w2 = qkwp.tile([128, NG * 64], MMDT, tag="w2")
                            nc.sync.dma_start(
                                w1[:], (qw1_d if ti == 0 else kw1_d)[:])
                            nc.sync.dma_start(
                                w2[:], (qw2_d if ti == 0 else kw2_d)[:])
                        else:
                            w0 = qkwp.tile([65, NG * 64], MMDT, tag="w0")
                            nc.sync.dma_start(
                                w0[:], (qwT_d if ti == 0 else kwT_d)[:])
                        for bank in range(16):
                            ps = qkpsp.tile([128, 512], F32, tag="ps")
                            for cc in range(4):
                                c = 4 * bank + cc
                                for half in range(2):
                                    g = 2 * c + half
                                    dst = ps[64 * half:64 * half + 64,
                                             128 * cc:128 * cc + 128]
                                    xcol = ti * 16384 + 128 * g
                                    if nobias:
                                        nc.tensor.matmul(
                                            dst, w1[:, 64 * g:64 * g + 64],
                                            xqk2[0:64, xcol:xcol + 128],
                                            start=True, stop=False)
                                        nc.tensor.matmul(
                                            dst, w2[:, 64 * g:64 * g + 64],
                                            xqk2[:, xcol:xcol + 128],
                                            start=False, stop=True)
                                    else:
                                        nc.tensor.matmul(
                                            dst, w0[:, 64 * g:64 * g + 64],
                                            xqk[:, xcol:xcol + 128],
                                            start=True, stop=True)
                            if eng is nc.scalar:
                                nc.scalar.copy(
                                    dstT[:, 512 * bank:512 * bank + 512], ps[:])
                            else:
                                nc.vector.tensor_copy(
                                    dstT[:, 512 * bank:512 * bank + 512], ps[:])

            # ---- scores + softmax for both batches (f32 matmuls) ----
            with tc.tile_pool(name="sm", bufs=1) as sm, \
                 tc.tile_pool(name="smps", bufs=2, space="PSUM") as smps:
                for b in range(B):
                    scps = smps.tile([64, 64], F32, tag="scps")
                    for c in range(64):
                        nc.tensor.matmul(
                            scps[:],
                            qT[:, 128 * c + 64 * b:128 * c + 64 * b + 64],
                            kT[:, 128 * c + 64 * b:128 * c + 64 * b + 64],
                            start=(c == 0), stop=(c == 63))
                    ssb = sm.tile([64, 64], F32, tag="ssb")
                    nc.vector.tensor_tensor(ssb[:], scps[:], battn[:], ALU.add)
                    nmax = sm.tile([64, 1], F32, tag="nmax")
                    nc.vector.tensor_reduce(nmax[:], ssb[:],
                                            mybir.AxisListType.X,
                                            ALU.max, negate=True)
                    expt = sm.tile([64, 64], F32, tag="expt")
                    sume = sm.tile([64, 1], F32, tag="sume")
                    nc.scalar.activation(expt[:], ssb[:], AF.Exp,
                                         bias=nmax[:], scale=1.0,
                                         accum_out=sume[:])
                    rsum = sm.tile([64, 1], F32, tag="rsum")
                    nc.vector.reciprocal(rsum[:], sume[:])
                    A_f = sm.tile([64, 64], F32, tag="A_f")
                    nc.vector.tensor_scalar_mul(A_f[:], expt[:], rsum[:])
                    atps = smps.tile([64, 64], F32, tag="atps")
                    nc.tensor.transpose(atps[:], A_f[:], id64[:])
                    nc.scalar.copy(AT[64 * b:64 * b + 64, :], atps[:])

        # ---------------- per-batch ----------------
        for b in range(B):
            with ExitStack() as bst:
                # ---- proj + LN1 ----
                mic1 = bst.enter_context(tc.tile_pool(name=f"mic1_{b}", bufs=1))
                x1pool = bst.enter_context(tc.tile_pool(name=f"x1_{b}", bufs=1))
                x1f = x1pool.tile([128, 4096], F32)
                x1b = x1pool.tile([128, 4096], MMDT)
                ln1_cm = tc.tile_pool(name=f"ln1_{b}", bufs=1)
                ln1 = ln1_cm.__enter__()
                projsb = ln1.tile([128, 4096], F32)
                stats6 = ln1.tile([128, 384], F32)
                with tc.tile_pool(name=f"pps{b}", bufs=2, space="PSUM") as ppsp:
                    for t in range(8):
                        pps = ppsp.tile([128, 512], F32, tag="pps")
                        for ww in range(16):
                            w = 16 * t + ww
                            rh = w % 2
                            qq = (w - 16 * t) // 2
                            nc.tensor.matmul(
                                pps[64 * rh:64 * rh + 64, 64 * qq:64 * qq + 64],
                                AT[64 * b:64 * b + 64, :],
                                vp[64 * b:64 * b + 64, 64 * w:64 * w + 64],
                                start=True, stop=True)
                        for qq in range(8):
                            nc.vector.bn_stats(
                                stats6[:, 48 * t + 6 * qq:48 * t + 6 * qq + 6],
                                pps[:, 64 * qq:64 * qq + 64])
                        nc.scalar.copy(projsb[:, 512 * t:512 * t + 512], pps[:])

                # batched LN1 micro-chain over 64 token-tiles
                s3 = stats6[:].rearrange("p (q s) -> p q s", s=6)
                me, cve = s3[:, :, 1:2], s3[:, :, 2:3]
                mo, cvo = s3[:, :, 4:5], s3[:, :, 5:6]
                m2x = mic1.tile([128, 64], F32)   # me+mo = 2*mean
                dd = mic1.tile([128, 64], F32)
                d2 = mic1.tile([128, 64], F32)
                cv = mic1.tile([128, 64], F32)
                v64 = mic1.tile([128, 64], F32)
                sig = mic1.tile([128, 64], F32)
                r1 = mic1.tile([128, 64], F32)
                nmr1 = mic1.tile([128, 64], F32)
                nc.vector.tensor_tensor(m2x[:], me, mo, ALU.add)
                nc.vector.tensor_tensor(dd[:], me, mo, ALU.subtract)
                nc.vector.tensor_tensor(d2[:], dd[:], dd[:], ALU.mult)
                nc.vector.tensor_tensor(cv[:], cve, cvo, ALU.add)
                # 64*var = (cve+cvo) + 16*(me-mo)^2
                nc.vector.scalar_tensor_tensor(v64[:], d2[:], 16.0, cv[:],
                                               ALU.mult, ALU.add)
                nc.scalar.activation(sig[:], v64[:], AF.Sqrt,
                                     bias=epsc[:], scale=1.0 / 64.0)
                nc.vector.reciprocal(r1[:], sig[:])
                nc.vector.scalar_tensor_tensor(nmr1[:], m2x[:], -0.5, r1[:],
                                               ALU.mult, ALU.mult)

                with tc.tile_pool(name=f"scp{b}", bufs=1) as scp:
                    scb = scp.tile([128, 4096], F32)
                    for t in range(8):
                        nc.sync.dma_start(
                            scb[:, 512 * t:512 * t + 512].rearrange(
                                "p (q c) -> p q c", c=64),
                            sc_v[b, t])
                    # normalize (split across ACT / DVE)
                    for q in range(64):
                        dst = x1f[:, 64 * q:64 * q + 64]
                        src = projsb[:, 64 * q:64 * q + 64]
                        if q % 2 == 0:
                            nc.scalar.activation(dst, src, AF.Identity,
                                                 bias=nmr1[:, q:q + 1],
                                                 scale=r1[:, q:q + 1])
                        else:
                            nc.vector.tensor_scalar(dst, src,
                                                    r1[:, q:q + 1],
                                                    nmr1[:, q:q + 1],
                                                    ALU.mult, ALU.add)
                        if has_g1:
                            nc.vector.tensor_tensor(dst, dst, g1bc[:], ALU.mult)
                    for t in range(8):
                        sl = slice(512 * t, 512 * t + 512)
                        nc.vector.tensor_tensor(x1f[:, sl], x1f[:, sl],
                                                scb[:, sl], ALU.add)
                        nc.scalar.copy(x1b[:, sl], x1f[:, sl])
                ln1_cm.__exit__(None, None, None)

                # ---- x1^T via PE transpose ----
                mlppool = bst.enter_context(
                    tc.tile_pool(name=f"mlp_{b}", bufs=1))
                x1T = mlppool.tile([64, 8192], MMDT)
                with tc.tile_pool(name=f"tps{b}", bufs=2, space="PSUM") as tpsp:
                    for tt in range(16):
                        tp = tpsp.tile([64, 512], MMDT, tag="tp")
                        for j in range(4):
                            q = 4 * tt + j
                            nc.tensor.transpose(tp[:, 128 * j:128 * j + 128],
                                                x1b[:, 64 * q:64 * q + 64],
                                                id128[:])
                        if tt % 2 == 0:
                            nc.vector.tensor_copy(x1T[:, 512 * tt:512 * tt + 512], tp[:])
                        else:
                            nc.scalar.copy(x1T[:, 512 * tt:512 * tt + 512], tp[:])

                # ---- fc1 + gelu -> hT (hid on partitions, bf16) ----
                hT = mlppool.tile([128, 16384], MMDT)  # [:, 8192*k + tok]
                with tc.tile_pool(name=f"f1ps{b}", bufs=2, space="PSUM") as f1p:
                    for k in range(2):
                        for blk in range(16):
                            fp = f1p.tile([128, 512], F32, tag="fp")
                            nc.tensor.matmul(
                                fp[:], fc1w[:, 128 * k:128 * k + 128],
                                x1T[:, 512 * blk:512 * blk + 512],
                                start=True, stop=True)
                            nc.scalar.activation(
                                hT[:, 8192 * k + 512 * blk:
                                   8192 * k + 512 * blk + 512],
                                fp[:], AF.Gelu,
                                bias=fc1b[:, k:k + 1], scale=1.0)

                # ---- fc2 + LN2 + residual + store ----
                mlpsb = mlppool.tile([128, 4096], F32)
                stats6b = mlppool.tile([128, 384], F32)
                final = mlppool.tile([128, 4096], F32)
                with tc.tile_pool(name=f"f2ps{b}", bufs=2, space="PSUM") as f2p:
                    for t in range(8):
                        mp = f2p.tile([128, 512], F32, tag="mp")
                        for gg in range(8):
                            blk = 8 * t + gg
                            for k in range(2):
                                nc.tensor.matmul(
                                    mp[:, 64 * gg:64 * gg + 64],
                                    hT[:, 8192 * k + 128 * blk:
                                       8192 * k + 128 * blk + 128],
                                    fc2w[:, 64 * k:64 * k + 64],
                                    start=(k == 0), stop=(k == 1))
                        if has_fc2b:
                            for gg in range(8):
                                nc.vector.tensor_tensor(
                                    mlpsb[:, 512 * t + 64 * gg:
                                          512 * t + 64 * gg + 64],
                                    mp[:, 64 * gg:64 * gg + 64],
                                    fc2bc[:], ALU.add)
                            for qq in range(8):
                                nc.vector.bn_stats(
                                    stats6b[:, 48 * t + 6 * qq:
                                            48 * t + 6 * qq + 6],
                                    mlpsb[:, 512 * t + 64 * qq:
                                          512 * t + 64 * qq + 64])
                        else:
                            for qq in range(8):
                                nc.vector.bn_stats(
                                    stats6b[:, 48 * t + 6 * qq:
                                            48 * t + 6 * qq + 6],
                                    mp[:, 64 * qq:64 * qq + 64])
                            nc.scalar.copy(mlpsb[:, 512 * t:512 * t + 512],
                                           mp[:])

                s3b = stats6b[:].rearrange("p (q s) -> p q s", s=6)
                me2, cve2 = s3b[:, :, 1:2], s3b[:, :, 2:3]
                mo2, cvo2 = s3b[:, :, 4:5], s3b[:, :, 5:6]
                m2x2 = mlppool.tile([128, 64], F32)
                dd2 = mlppool.tile([128, 64], F32)
                dsq2 = mlppool.tile([128, 64], F32)
                cv2 = mlppool.tile([128, 64], F32)
                v642 = mlppool.tile([128, 64], F32)
                sig2 = mlppool.tile([128, 64], F32)
                r2 = mlppool.tile([128, 64], F32)
                nmr2 = mlppool.tile([128, 64], F32)
                nc.vector.tensor_tensor(m2x2[:], me2, mo2, ALU.add)
                nc.vector.tensor_tensor(dd2[:], me2, mo2, ALU.subtract)
                nc.vector.tensor_tensor(dsq2[:], dd2[:], dd2[:], ALU.mult)
                nc.vector.tensor_tensor(cv2[:], cve2, cvo2, ALU.add)
                nc.vector.scalar_tensor_tensor(v642[:], dsq2[:], 16.0, cv2[:],
                                               ALU.mult, ALU.add)
                nc.scalar.activation(sig2[:], v642[:], AF.Sqrt,
                                     bias=epsc[:], scale=1.0 / 64.0)
                nc.vector.reciprocal(r2[:], sig2[:])
                nc.vector.scalar_tensor_tensor(nmr2[:], m2x2[:], -0.5, r2[:],
                                               ALU.mult, ALU.mult)

                for q in range(64):
                    dst = final[:, 64 * q:64 * q + 64]
                    src = mlpsb[:, 64 * q:64 * q + 64]
                    if q % 2 == 0:
                        nc.scalar.activation(dst, src, AF.Identity,
                                             bias=nmr2[:, q:q + 1],
                                             scale=r2[:, q:q + 1])
                    else:
                        nc.vector.tensor_scalar(dst, src,
                                                r2[:, q:q + 1],
                                                nmr2[:, q:q + 1],
                                                ALU.mult, ALU.add)
                    if has_g2:
                        nc.vector.tensor_tensor(dst, dst, g2bc[:], ALU.mult)
                    if has_n2b:
                        nc.vector.tensor_tensor(dst, dst, n2bc[:], ALU.add)
                for t in range(8):
                    sl = slice(512 * t, 512 * t + 512)
                    nc.vector.tensor_tensor(final[:, sl], final[:, sl],
                                            x1f[:, sl], ALU.add)
                    nc.sync.dma_start(
                        out_v[b, t],
                        final[:, sl].rearrange("p (q c) -> p q c", c=64))

    nc.compile()
    return nc


def _host_prep(inputs, core):
    x = _f32(np.asarray(inputs['x']))
    qkv_w = _f32(np.asarray(inputs['qkv_w']))
    qkv_b = _f32(np.asarray(inputs['qkv_b']))
    proj_w = _f32(np.asarray(inputs['proj_w']))
    proj_b = _f32(np.asarray(inputs['proj_b']))
    rpb = _f32(np.asarray(inputs['rpb_table']))
    n1b = _f32(np.asarray(inputs['norm1_b']))
    fc1_w = _f32(np.asarray(inputs['fc1_w']))
    fc1_b = _f32(np.asarray(inputs['fc1_b']))
    fc2_w = _f32(np.asarray(inputs['fc2_w']))

    i = core
    xw = x.reshape(B, 32, 8, 32, 8, C).transpose(0, 1, 3, 2, 4, 5) \
          .reshape(B, NW, S, C)

    nobias = bool(np.all(qkv_b == 0.0) and np.all(proj_b == 0.0))

    # col = g*128 + b*64 + s within each section
    ch0 = (np.arange(3)[:, None] * 65536 + i * D
           + 64 * np.arange(NG)[None, :])          # (3, NG)
    wins = ch0 // 192                               # (3, NG)
    joff = ch0 % 192
    xsel = xw[:, wins]                              # (B, 3, NG, S, C)
    xflat = xsel.transpose(4, 1, 2, 0, 3).reshape(64, 3 * 16384)

    xTv = np.empty((65, 16384), np.float32)
    xTv[:64] = xflat[:, 2 * 16384:]
    xTv[64] = 1.0

    def wslices(t, scale, fold_proj):
        W = np.empty((65, NG * 64), np.float32)
        for g in range(NG):
            j = int(joff[t, g])
            sl = qkv_w[j:j + 64, :]
            bb = qkv_b[j:j + 64]
            if fold_proj:
                W[:64, 64 * g:64 * g + 64] = sl.T @ proj_w.T
                W[64, 64 * g:64 * g + 64] = proj_w @ bb + proj_b
            else:
                W[:64, 64 * g:64 * g + 64] = sl.T * scale
                W[64, 64 * g:64 * g + 64] = bb * scale
        return W

    rel = _rel_index()
    battn = rpb[rel.reshape(-1)].reshape(S, S, NH)[:, :, i]

    sc = xw[:, 128 * i:128 * (i + 1)].reshape(B, 8192, C) + n1b

    m = {
        "xTv": _bf(xTv),
        "vpW": _bf(wslices(2, 1.0, True)),
        "id128": _bf(np.eye(128)),
        "id64": _f32(np.eye(64)),
        "battn": _f32(battn),
        "fc1wT": _bf(fc1_w.T),
        "fc1b": _f32(fc1_b.reshape(2, 128).T),
        "fc2wT": _bf(fc2_w.T.reshape(2, 128, 64).transpose(1, 0, 2)
                     .reshape(128, 128)),
        "sc": _f32(sc),
    }
    xqk_f32 = xflat[:, 0:2 * 16384]
    if nobias:
        xh = xqk_f32.astype(MMNP)
        xl = (xqk_f32 - xh.astype(np.float32)).astype(MMNP)
        xqk2 = np.concatenate([xh, xl], axis=0)     # (128, 32768)
        m["xqk2"] = np.ascontiguousarray(xqk2)
        for nm, t, scale in (("qw", 0, SCALE), ("kw", 1, 1.0)):
            Wf = wslices(t, scale, False)[:64]       # (64, NG*64) f32
            Wh = Wf.astype(MMNP)
            Wl = (Wf - Wh.astype(np.float32)).astype(MMNP)
            m[nm + "1"] = np.ascontiguousarray(Wh)
            m[nm + "2"] = np.ascontiguousarray(
                np.concatenate([Wl, Wh], axis=0))    # (128, NG*64)
    else:
        xqk = np.empty((65, 2 * 16384), np.float32)
        xqk[:64] = xqk_f32
        xqk[64] = 1.0
        m["xqk"] = _bf(xqk)
        m["qwT"] = _bf(wslices(0, SCALE, False))
        m["kwT"] = _bf(wslices(1, 1.0, False))
    return m


def _flags(inputs):
    n1g = np.asarray(inputs['norm1_g'])
    n2g = np.asarray(inputs['norm2_g'])
    n2b = np.asarray(inputs['norm2_b'])
    fc2_b = np.asarray(inputs['fc2_b'])
    nobias = bool(np.all(np.asarray(inputs['qkv_b']) == 0.0)
                  and np.all(np.asarray(inputs['proj_b']) == 0.0))
    return (nobias, not np.all(n1g == 1.0), not np.all(n2g == 1.0),
            not np.all(n2b == 0.0), not np.all(fc2_b == 0.0))


def kernel(**inputs):
    flags = _flags(inputs)
    if flags not in _BUILD_CACHE:
        _BUILD_CACHE[flags] = _build(flags)
    nc = _BUILD_CACHE[flags]

    in_maps = []
    for i in range(NCORES):
        m = _host_prep(inputs, i)
        nobias, has_g1, has_g2, has_n2b, has_fc2b = flags
        if has_g1:
            m["g1bc"] = _f32(np.tile(np.asarray(inputs['norm1_g']), (128, 1)))
        if has_g2:
            m["g2bc"] = _f32(np.tile(np.asarray(inputs['norm2_g']), (128, 1)))
        if has_n2b:
            m["n2bc"] = _f32(np.tile(np.asarray(inputs['norm2_b']), (128, 1)))
        if has_fc2b:
            m["fc2bc"] = _f32(np.tile(np.asarray(inputs['fc2_b']), (128, 1)))
        in_maps.append(m)

    res = run_bass_kernel_spmd(nc, in_maps, list(range(NCORES)))

    owin = np.empty((B, NW, S, C), np.float32)
    for i in range(NCORES):
        owin[:, 128 * i:128 * (i + 1)] = \
            res.results[i]["out"].reshape(B, 128, S, C)
    out = owin.reshape(B, 32, 32, 8, 8, C).transpose(0, 1, 3, 2, 4, 5) \
              .reshape(B, 256 * 256, C)
    return out.astype(np.asarray(inputs['x']).dtype)
